# revision 14
# baseline (speedup 1.0000x reference)
"""DiT block kernel for 8 TRN2 NeuronCores (self-contained).

Sharding: cores 0-3 <-> batch 0, cores 4-7 <-> batch 1.
Per 4-core group: attention head-parallel (3 of 12 heads/core, all 2048
tokens), W_o row-sharded -> per-quarter ReduceScatter (4x bf16) -> each
core owns 4x128-token slices; FFN token-parallel (512 rows, fp8 weights
prefetched to SBUF during attention). AdaLN/cond path is DH-sharded over
all 8 cores with host-folded (cond_w2 @ W_mod) matrices -> two small
AllReduces overlapped with the rms-stats phase (a dummy AllReduce at t=0
absorbs the collective entry barrier).

The attention-norm modulation is folded into the QKV weights on-chip:
wqk/wv rows are scaled by gamma after the AllReduce lands, the beta term
is injected as a rank-1 matmul (beta^T W (x) rms), and the 1/rms factor
is applied to the matmul outputs (rows for q/k via a broadcast tile,
columns for v via per-partition scalars). This removes the full-width
modulated-h pass. Attention runs with 1024-wide double-buffered score
groups: 2 score matmuls -> one wide exp -> 2 PV matmuls, so the PE
stream never blocks on the ScalarE exp. Softmax denominators use the
ones-augmented V trick; reciprocals are batched [3,512] per q-tile.
"""
import numpy as np
import ml_dtypes

import concourse.bass as bass
import concourse.mybir as mybir
import concourse.tile as tile
from concourse import bacc, bass_utils
from concourse.masks import make_identity

FP32 = mybir.dt.float32
FP32R = mybir.dt.float32r
BF16 = mybir.dt.bfloat16
FP8E4 = mybir.dt.float8e4
PM_DR = mybir.MatmulPerfMode.DoubleRow
AF = mybir.ActivationFunctionType
ALU = mybir.AluOpType
AX = mybir.AxisListType

NPBF = ml_dtypes.bfloat16

B, L, D, H, DH = 2, 2048, 768, 12, 3072
HD = 64
EPS = 1e-6
SCALE = float(np.sqrt(HD))
NC_ = 8
G = 4            # cores per batch group
HC = 3           # heads per core
TOK = L // G     # 512
QTOK = 128       # tokens per core per quarter
DH8 = DH // NC_  # 384
GROUPS = [[0, 1, 2, 3], [4, 5, 6, 7]]
KC = L // 128    # 16 key chunks
JT = L // 512    # 4 q tiles
DK = D // 128    # 6 d chunks
MG = DH // 128   # 24 dh chunks


def _bf(a):
    return np.ascontiguousarray(np.asarray(a, np.float32)).astype(NPBF)


NPF8 = ml_dtypes.float8_e4m3


def _f8(a, scale):
    a = np.asarray(a, np.float32) * scale
    return np.ascontiguousarray(np.clip(a, -240.0, 240.0)).astype(NPF8)


# ---------------------------------------------------------------- host prep
def host_prep(inp):
    f = {k: np.ascontiguousarray(np.asarray(v, np.float32)) for k, v in inp.items()}
    x, c = f["x"], f["c"]
    cos, sin = f["freqs_cos"], f["freqs_sin"]          # [L, 32]

    attn_gamma_s = f["attn_gamma"] * f["attn_norm_w"][None, :]
    ffn_gamma_s = f["ffn_gamma"] * f["ffn_norm_w"][None, :]
    mods = [attn_gamma_s, f["attn_beta"], f["attn_alpha"],
            ffn_gamma_s, f["ffn_beta"], f["ffn_gamma"]]
    wfold_full = [f["cond_w2"] @ m for m in mods]       # [DH, D] x6
    bvec = np.stack([f["cond_b2"] @ m for m in mods])   # [6, D]
    # FFN norm/modulation path carries an extra x8 so h2 lands in fp8
    # e4m3's normal range; gate/hidden weights carry x32. The product
    # 8*32=256 is divided back out after the gate/hidden matmuls.
    for mi in (3, 4):
        wfold_full[mi] = wfold_full[mi] * 8.0
        bvec[mi] = bvec[mi] * 8.0

    perm = np.concatenate([np.arange(0, HD, 2), np.arange(1, HD, 2)])
    cosT, sinT = cos.T, sin.T                            # [32, L]
    cct = np.tile(cosT, (4, 1)).astype(np.float32)       # [128, L]
    sst = np.concatenate([-sinT, sinT, -sinT, sinT], 0).astype(np.float32)

    cT = np.ascontiguousarray(c.T)                       # [768, 2]
    ct_pack = cT.reshape(6, 128, 2).transpose(1, 0, 2).reshape(128, 12).copy()

    wg_blk = f["ffn_gate"].reshape(6, 128, 24, 128).transpose(2, 1, 0, 3) \
        .reshape(24 * 128, 768)
    wh_blk = f["ffn_hidden"].reshape(6, 128, 24, 128).transpose(2, 1, 0, 3) \
        .reshape(24 * 128, 768)
    # [dh-in-chunk 128, dh-chunk 24, d 768] for DoubleRow rhs slices
    wout2 = f["ffn_out"].reshape(24, 128, 768).transpose(1, 0, 2) \
        .reshape(128, 24 * 768)

    pswap = np.zeros((128, 128), np.float32)
    for i_ in range(128):
        pswap[i_, i_ ^ 32] = 1.0

    cores = []
    for i in range(NC_):
        g, r = i // G, i % G
        hs = [HC * r + j for j in range(HC)]
        si = slice(DH8 * i, DH8 * (i + 1))
        blocks = [f["W_q"][:, h * HD:(h + 1) * HD][:, perm] for h in hs]
        blocks += [f["W_k"][:, h * HD:(h + 1) * HD][:, perm] for h in hs]
        wqk = np.concatenate(blocks, 1)                  # [768, 384]
        wv = np.concatenate(
            [f["W_v"][:, h * HD:(h + 1) * HD] for h in hs], 1)  # [768, 192]
        wo = np.concatenate([f["W_o"][h * HD:(h + 1) * HD] for h in hs], 0)

        # token ownership: 4 slices of 128, one per query quarter
        x_slice = np.concatenate(
            [x[g, 512 * t + QTOK * r: 512 * t + QTOK * (r + 1)]
             for t in range(4)], 0)                      # [512, 768]

        gsel = np.zeros((2, 1), np.float32)
        gsel[g, 0] = 1.0

        cores.append(dict(
            xT=_bf(x[g].T),
            x_slice=_bf(x_slice),
            cct=_bf(cct), sst=_bf(sst),
            ct_pack=_bf(ct_pack),
            w1s=_bf(f["cond_w1"][:, si]),
            b1_rows=np.ascontiguousarray(
                np.tile(f["cond_b1"][si][None, :], (2, 1))),    # [2, 384]
            wfold=_bf(np.concatenate([w[si] for w in wfold_full], 0)),
            gsel=_bf(gsel),
            bv_cat=np.ascontiguousarray(bvec.reshape(1, 6 * D)),
            pswap=_bf(pswap),
            wqk=_bf(wqk), wv=_bf(wv), wo=_bf(wo),
            wg_blk=_f8(wg_blk, 32.0), wh_blk=_f8(wh_blk, 32.0),
            wout2=_f8(wout2, 32.0),
        ))
    return cores


# ---------------------------------------------------------------- program
_CACHE = {}

DRAM_SPECS = [
    ("xT", [D, L], BF16),
    ("x_slice", [TOK, D], BF16),
    ("cct", [128, L], BF16),
    ("sst", [128, L], BF16),
    ("ct_pack", [128, 12], BF16),
    ("w1s", [D, DH8], BF16),
    ("b1_rows", [2, DH8], FP32),
    ("wfold", [6 * DH8, D], BF16),
    ("gsel", [2, 1], BF16),
    ("bv_cat", [1, 6 * D], FP32),
    ("pswap", [128, 128], BF16),
    ("wqk", [D, 384], BF16),
    ("wv", [D, HC * HD], BF16),
    ("wo", [HC * HD, D], BF16),
    ("wg_blk", [MG * 128, D], FP8E4),
    ("wh_blk", [MG * 128, D], FP8E4),
    ("wout2", [128, MG * D], FP8E4),
]


def build_program(reps=1):
    nc = bacc.Bacc("TRN2", target_bir_lowering=False, debug=False,
                   num_devices=NC_)
    dr = {}
    for name, shape, dt in DRAM_SPECS:
        dr[name] = nc.dram_tensor(name, shape, dt, kind="ExternalInput")
    out_d = nc.dram_tensor("out", [TOK, D], FP32, kind="ExternalOutput")

    with tile.TileContext(nc) as tc, \
         nc.allow_low_precision(reason="bf16 matmuls, fp32 PSUM accumulation"):
        for _ in range(reps):
            _emit(nc, tc, dr, out_d)
    nc.compile()
    return nc


def _phase_a1(nc, tc, dr, st):
    """cond MLP partials (row layout); AR for mods 0,1 issued early."""
    with tc.tile_pool(name="pa", bufs=1) as pa, \
         tc.tile_pool(name="pa_wf", bufs=6) as pa_wf, \
         tc.tile_pool(name="pa_ps", bufs=2, space="PSUM") as pa_ps:
        ct_sb = pa.tile([128, 12], BF16, name="ct_sb")
        nc.sync.dma_start(ct_sb[:], dr["ct_pack"].ap())
        b1_sb = pa.tile([2, DH8], FP32, name="b1_sb")
        nc.sync.dma_start(b1_sb[:], dr["b1_rows"].ap())
        w1_sb = [pa.tile([128, DH8], BF16, name=f"w1_sb{k}")
                 for k in range(DK)]
        for k in range(DK):
            nc.sync.dma_start(w1_sb[k][:],
                              dr["w1s"].ap()[128 * k:128 * (k + 1), :])

        # h1 [2, 384] = c @ W1s  (both batches)
        h1_ps = pa_ps.tile([2, DH8], FP32, name="h1_ps")
        for k in range(DK):
            nc.tensor.matmul(h1_ps[:], ct_sb[:, 2 * k:2 * k + 2],
                             w1_sb[k][:],
                             start=(k == 0), stop=(k == DK - 1))
        h1_sb = pa.tile([2, DH8], FP32, name="h1_sb")
        nc.vector.tensor_add(h1_sb[:], h1_ps[:], b1_sb[:])
        silu_r = pa.tile([2, DH8], BF16, name="silu_r")
        nc.scalar.activation(silu_r[:], h1_sb[:], AF.Silu)
        # silu columns [128, 2] x3 chunks
        silu_c = pa.tile([128, 6], BF16, name="silu_c")
        scp = pa_ps.tile([128, 6], BF16, name="scp")
        for cch in range(3):
            nc.tensor.transpose(scp[:, 2 * cch:2 * cch + 2],
                                silu_r[:, 128 * cch:128 * (cch + 1)],
                                st["ident_bf"][0:2, 0:2])
        nc.vector.tensor_copy(silu_c[:], scp[:])

        arin_a = pa.tile([2, 2 * D], FP32, name="arin_a")
        arin_b = pa.tile([2, 4 * D], FP32, name="arin_b")
        for m in range(6):
            for half in range(2):
                mp = pa_ps.tile([2, 384], FP32, name="mp")
                for k in range(3):
                    wf = pa_wf.tile([128, D], BF16, name="wf")
                    if half == 0:
                        nc.sync.dma_start(
                            wf[:], dr["wfold"].ap()[128 * (3 * m + k):
                                                    128 * (3 * m + k + 1), :])
                        st[f"wf{m}_{k}"] = wf
                    else:
                        wf = st[f"wf{m}_{k}"]
                    nc.tensor.matmul(mp[:],
                                     silu_c[:, 2 * k:2 * k + 2],
                                     wf[:, 384 * half:384 * (half + 1)],
                                     start=(k == 0), stop=(k == 2))
                if m < 2:
                    dst = arin_a[0:2, m * D + 384 * half:
                                 m * D + 384 * (half + 1)]
                else:
                    dst = arin_b[0:2, (m - 2) * D + 384 * half:
                                 (m - 2) * D + 384 * (half + 1)]
                nc.vector.tensor_copy(dst, mp[:])
            if m == 1:
                nc.sync.dma_start(st["ar_a_in"][:], arin_a[:])
                nc.gpsimd.collective_compute(
                    "AllReduce", ALU.add,
                    replica_groups=[list(range(NC_))],
                    ins=[st["ar_a_in"].opt()], outs=[st["ar_a_out"].opt()])
        nc.sync.dma_start(st["ar_b_in"][:], arin_b[:])
        nc.gpsimd.collective_compute(
            "AllReduce", ALU.add, replica_groups=[list(range(NC_))],
            ins=[st["ar_b_in"].opt()], outs=[st["ar_b_out"].opt()])


def _mod_row(nc, st, pa2_ps, ar_bf, bv_sb, mod_bf, base_col, m):
    """One mod row [1, D] at partition 0 from the reduced tensor."""
    for half in range(2):
        sl = slice(384 * half, 384 * (half + 1))
        sp = pa2_ps.tile([1, 384], FP32, name="sp")
        nc.tensor.matmul(sp[:], st["gsel_sb"][:],
                         ar_bf[0:2, base_col + 384 * half:
                               base_col + 384 * (half + 1)])
        nc.vector.tensor_add(mod_bf[0:1, sl], sp[:],
                             bv_sb[0:1, m * D + 384 * half:
                                   m * D + 384 * (half + 1)])


def _phase_a2_cols(nc, tc, dr, st):
    """Consume AR-a -> mod_cols (gamma/beta for attention norm)."""
    with tc.tile_pool(name="pa2c", bufs=1) as pa2, \
         tc.tile_pool(name="pa2c_ps", bufs=2, space="PSUM") as pa2_ps:
        ar_sb = pa2.tile([2, 2 * D], FP32, name="ara_sb")
        nc.sync.dma_start(ar_sb[:], st["ar_a_out"][:])
        ar_bf = pa2.tile([2, 2 * D], BF16, name="ara_bf")
        nc.vector.tensor_copy(ar_bf[:], ar_sb[:])
        bv_sb = pa2.tile([1, 2 * D], FP32, name="bv_a")
        nc.sync.dma_start(bv_sb[:], dr["bv_cat"].ap()[0:1, 0:2 * D])
        mod_bf = [pa2.tile([1, D], BF16, name=f"mod_ab{m}") for m in range(2)]
        for m in range(2):
            _mod_row(nc, st, pa2_ps, ar_bf, bv_sb, mod_bf[m], m * D, m)
        # even psum cols so each bf16 write stays 4-byte aligned
        mcp = pa2_ps.tile([128, 24], BF16, name="mcp")
        for m in range(2):
            for k in range(DK):
                col = 2 * (6 * m + k)
                nc.tensor.transpose(mcp[:, col:col + 1],
                                    mod_bf[m][0:1, 128 * k:128 * (k + 1)],
                                    st["ident_bf"][0:1, 0:1])
        nc.vector.tensor_copy(st["mod_cols"][:], mcp[:, 0:24:2])
        nc.vector.tensor_copy(st["mod_cols_bf"][:], mcp[:, 0:24:2])


def _phase_a2_bcast(nc, tc, dr, st):
    """Consume AR-b -> bcast tiles for mods 2..5."""
    bcast = st["bcast"]
    with tc.tile_pool(name="pa2b", bufs=1) as pa2, \
         tc.tile_pool(name="pa2b_ps", bufs=2, space="PSUM") as pa2_ps:
        ar_sb = pa2.tile([2, 4 * D], FP32, name="arb_sb")
        nc.sync.dma_start(ar_sb[:], st["ar_b_out"][:])
        ar_bf = pa2.tile([2, 4 * D], BF16, name="arb_bf")
        nc.vector.tensor_copy(ar_bf[:], ar_sb[:])
        bv_sb = pa2.tile([1, 6 * D], FP32, name="bv_b")
        nc.sync.dma_start(bv_sb[:], dr["bv_cat"].ap())
        for m in (2, 3, 4, 5):
            mod_bf = pa2.tile([1, D], BF16, name=f"mod_bb{m}")
            _mod_row(nc, st, pa2_ps, ar_bf, bv_sb, mod_bf, (m - 2) * D, m)
            for half in range(2):
                sl = slice(384 * half, 384 * (half + 1))
                bp = pa2_ps.tile([128, 384], FP32, name="bp")
                nc.tensor.matmul(bp[:], st["ones_bf"][:],
                                 mod_bf[0:1, sl])
                nc.vector.tensor_copy(bcast[m][:, sl], bp[:])


def _phase_b_stats(nc, tc, dr, st, xt):
    """xT load + rms stats -> rms rows (sq_r), 1/rms rows (rinv_r),
    broadcast 1/rms tile rb [128,L], and column-form rb_cols [128,16]."""
    with tc.tile_pool(name="pb", bufs=2) as pb, \
         tc.tile_pool(name="pb_ps", bufs=1, space="PSUM") as pb_ps:
        for k in range(DK):
            nc.sync.dma_start(xt[k][:],
                              dr["xT"].ap()[128 * k:128 * (k + 1), :])
        msq = [pb_ps.tile([1, 512], FP32, name=f"msq{j}") for j in range(4)]
        for k in range(DK):
            for j in range(4):
                xsq = pb.tile([128, 512], BF16, name="xsq")
                nc.vector.tensor_mul(xsq[:], xt[k][:, 512 * j:512 * (j + 1)],
                                     xt[k][:, 512 * j:512 * (j + 1)])
                nc.tensor.matmul(msq[j][:], st["onescol_bf"][:], xsq[:],
                                 start=(k == 0), stop=(k == DK - 1))
        for j in range(4):
            nc.scalar.activation(st["sq_j"][j][:], msq[j][:], AF.Sqrt,
                                 bias=st["eps_sb"][0:1, 0:1], scale=1.0 / D)
            nc.vector.reciprocal(st["rinv_j"][j][:], st["sq_j"][j][:])
            rbp = pb_ps.tile([128, 512], FP32, name="rbp", bufs=2)
            nc.tensor.matmul(rbp[:], st["ones_r"][:], st["rinv_j"][j][:])
            nc.vector.tensor_copy(st["rb"][:, 512 * j:512 * (j + 1)], rbp[:])
        # column-form 1/rms for the v path: 16 tiny transposes off the
        # bf16 broadcast tile's row 0 (even psum cols keep bf16 writes
        # 4-byte aligned)
        rcp = pb_ps.tile([128, 32], BF16, name="rcp")
        for t in range(KC):
            nc.tensor.transpose(
                rcp[:, 2 * t:2 * t + 1],
                st["rb"][0:1, 128 * t:128 * (t + 1)],
                st["ident_bf"][0:1, 0:1])
        nc.vector.tensor_copy(st["rb_cols"][:], rcp[:, 0:32:2])


def _phase_wmod(nc, tc, st, wqk_sb, wv_sb):
    """beta^T W rows (rank-1 bias operands) + in-place gamma row-scaling
    of the QKV weight tiles. Runs right after AR-a lands."""
    with tc.tile_pool(name="pwm_ps", bufs=2, space="PSUM") as pwm_ps:
        bq_ps = pwm_ps.tile([1, 384], FP32, name="bq_ps")
        for k in range(DK):
            nc.tensor.matmul(bq_ps[:], st["mod_cols_bf"][:, 6 + k:7 + k],
                             wqk_sb[k][:],
                             start=(k == 0), stop=(k == DK - 1))
        nc.scalar.activation(st["bq_r"][:], bq_ps[:], AF.Copy)
        bv_ps = pwm_ps.tile([1, HC * HD], FP32, name="bv_ps")
        for k in range(DK):
            nc.tensor.matmul(bv_ps[:], st["mod_cols_bf"][:, 6 + k:7 + k],
                             wv_sb[k][:],
                             start=(k == 0), stop=(k == DK - 1))
        nc.scalar.activation(st["bvw_r"][:], bv_ps[:], AF.Copy)
        for k in range(DK):
            nc.vector.tensor_scalar(wqk_sb[k][:], wqk_sb[k][:],
                                    st["mod_cols"][:, k:k + 1], None,
                                    op0=ALU.mult)
            nc.vector.tensor_scalar(wv_sb[k][:], wv_sb[k][:],
                                    st["mod_cols"][:, k:k + 1], None,
                                    op0=ALU.mult)


def _phase_c(nc, tc, dr, st, xt, qkr, v_sb, wqk_sb, wv_sb):
    """QKV matmuls on raw xT with folded modulation + RoPE + v tiles."""
    with tc.tile_pool(name="pc_w", bufs=1) as pc_w, \
         tc.tile_pool(name="pc", bufs=2) as pc, \
         tc.tile_pool(name="pc_ps", bufs=2, space="PSUM") as pc_ps:
        cct_sb = pc_w.tile([128, L], BF16, name="cct_sb")
        sst_sb = pc_w.tile([128, L], BF16, name="sst_sb")
        nc.sync.dma_start(cct_sb[:], dr["cct"].ap())
        nc.sync.dma_start(sst_sb[:], dr["sst"].ap())
        pswap_sb = pc_w.tile([128, 128], BF16, name="pswap_sb")
        nc.sync.dma_start(pswap_sb[:], dr["pswap"].ap())

        for m in range(3):
            qk_sb_m = pc.tile([128, L], BF16, name="qk_sb", bufs=2)
            for j in range(4):
                qkp = pc_ps.tile([128, 512], FP32, name="qkp")
                # rank-1: (Wq^T beta) (x) rms; the later 1/rms multiply
                # turns it into the plain beta bias term.
                nc.tensor.matmul(qkp[:], st["bq_r"][0:1, 128 * m:128 * (m + 1)],
                                 st["sq_j"][j][:],
                                 start=True, stop=False)
                for k in range(DK):
                    nc.tensor.matmul(qkp[:],
                                     wqk_sb[k][:, 128 * m:128 * (m + 1)],
                                     xt[k][:, 512 * j:512 * (j + 1)],
                                     start=False, stop=(k == DK - 1))
                nc.vector.tensor_mul(qk_sb_m[:, 512 * j:512 * (j + 1)],
                                     qkp[:], st["rb"][:, 512 * j:512 * (j + 1)])
            t1 = pc.tile([128, L], BF16, name="t1", bufs=1)
            t2 = pc.tile([128, L], BF16, name="t2", bufs=1)
            nc.vector.tensor_mul(t1[:], qk_sb_m[:], cct_sb[:])
            for j in range(4):
                sl = slice(512 * j, 512 * (j + 1))
                swp = pc_ps.tile([128, 512], FP32, name="swp")
                nc.tensor.matmul(swp[:], pswap_sb[:], qk_sb_m[:, sl])
                nc.vector.tensor_mul(t2[:, sl], swp[:], sst_sb[:, sl])
            qkr_A, qkr_B, qkr_C, qkr_D, qkr_E, qkr_F = qkr
            if m == 0:      # (q0, q1) -> A full
                nc.vector.tensor_add(qkr_A[:], t1[:], t2[:])
            elif m == 1:    # (q2, k0) -> C[0:64], B[0:64]; q2 dup -> E[64:]
                nc.vector.tensor_add(qkr_C[:], t1[0:64, :], t2[0:64, :])
                nc.vector.tensor_add(qkr_E[64:128, :], t1[0:64, :],
                                     t2[0:64, :])
                nc.vector.tensor_add(qkr_B[0:64, :], t1[64:128, :],
                                     t2[64:128, :])
            else:           # (k1, k2) -> B[64:128], D[0:64]; k2 dup -> F[64:]
                nc.vector.tensor_add(qkr_B[64:128, :], t1[0:64, :],
                                     t2[0:64, :])
                nc.vector.tensor_add(qkr_D[:], t1[64:128, :],
                                     t2[64:128, :])
                nc.vector.tensor_add(qkr_F[64:128, :], t1[64:128, :],
                                     t2[64:128, :])

        for t in range(KC):
            vp = pc_ps.tile([128, HC * HD], FP32, name="vp")
            # rank-1 beta term (x) rms rows; divided out by rb_cols below
            nc.tensor.matmul(
                vp[:],
                st["sq_j"][t // 4][0:1, 128 * (t % 4):128 * (t % 4) + 128],
                st["bvw_r"][:], start=True, stop=False)
            for k in range(DK):
                nc.tensor.matmul(vp[:], xt[k][:, 128 * t:128 * (t + 1)],
                                 wv_sb[k][:],
                                 start=False, stop=(k == DK - 1))
            for h in range(HC):
                nc.vector.tensor_scalar(
                    v_sb[t // 2][:, t % 2, 80 * h:80 * h + 64],
                    vp[:, 64 * h:64 * (h + 1)],
                    st["rb_cols"][:, t:t + 1], None, op0=ALU.mult)
            nc.scalar.activation(v_sb[t // 2][:, t % 2, 64:225:80],
                                 st["ones3_f"][:], AF.Copy)


def _ffn_prep_closures(nc, tc, st, x1, half, pf):
    """Residual add + rms stats + modulation for a 256-token half, on DVE.
    Returns (closures, h2cs dict filled as closures run)."""
    bcast = st["bcast"]
    ts_ = (0, 1) if half == 0 else (2, 3)
    h2cs = {}
    ms = pf.tile([128, 2], FP32, name=f"ms{half}")
    rv = pf.tile([128, 2], FP32, name=f"rv{half}")

    def stat(idx, t):
        def run():
            rsx = pf.tile([128, D], BF16, name="rsx")
            nc.sync.dma_start(rsx[:], st[f"rs_out{t}"][:])
            nc.vector.tensor_add(x1[t][:], x1[t][:], rsx[:])
            sq = pf.tile([128, D], BF16, name="sq")
            nc.vector.tensor_mul(sq[:], x1[t][:], x1[t][:])
            nc.vector.reduce_sum(ms[:, idx:idx + 1], sq[:], axis=AX.X)
        return run

    def rsq():
        sr = pf.tile([128, 2], FP32, name=f"sr{half}")
        nc.scalar.activation(sr[:], ms[:], AF.Sqrt,
                             bias=st["eps_sb"][:, 0:1], scale=1.0 / D)
        nc.vector.reciprocal(rv[:], sr[:])

    def modt(idx, t):
        def run():
            h2a = pf.tile([128, D], BF16, name="h2a")
            nc.vector.tensor_scalar(h2a[:], x1[t][:], rv[:, idx:idx + 1],
                                    None, op0=ALU.mult)
            h2m = pf.tile([128, D], BF16, name="h2m")
            nc.vector.tensor_mul(h2m[:], h2a[:], bcast[3][:])
            h2c = pf.tile([128, D], BF16, name=f"h2c{t}")
            nc.vector.tensor_add(h2c[:], h2m[:], bcast[4][:])
            h2cs[t] = h2c
        return run

    closures = [stat(0, ts_[0]), stat(1, ts_[1]), rsq,
                modt(0, ts_[0]), modt(1, ts_[1])]
    return closures, h2cs


def _ffn_tr_closures(nc, tc, st, h2f, half, h2cs, trps):
    """Transpose h2c -> h2f (fp8, plane-blocked); 4 closures of 3 chunks."""
    ident = st["ident_bf"]
    ts_ = (0, 1) if half == 0 else (2, 3)
    out = []
    for t in ts_:
        for half_k in range(2):
            ks = range(3 * half_k, 3 * half_k + 3)

            def run(t=t, ks=ks):
                for k in ks:
                    tp = trps.tile([128, 128], BF16, name="trp", tag="sh")
                    nc.tensor.transpose(tp[:],
                                        h2cs[t][:, 128 * k:128 * (k + 1)],
                                        ident[:])
                    nc.vector.tensor_copy(h2f[:, k, 128 * t:128 * (t + 1)],
                                          tp[:])
            out.append(run)
    return out


def _ffn_gatehid(nc, tc, st, h2f, half):
    """Gate/hidden fp8 DoubleRow matmuls + SwiGLU for one token half.
    Weights carry x32, h2 carries x8 -> psum is 256x; divided out in
    the silu scale and the ghF write (which re-applies x8 for fp8)."""
    sl = slice(256 * half, 256 * (half + 1))
    pfg = tc.alloc_tile_pool(name=f"pfg{half}", bufs=2)
    pf_gps = tc.alloc_tile_pool(name=f"pfg{half}_ps", bufs=2, space="PSUM")
    for m in range(MG):
        gp = pf_gps.tile([128, 256], FP32, name="gp")
        hp = pf_gps.tile([128, 256], FP32, name="hp")
        for a in range(DK // 2):
            nc.tensor.matmul(gp[:], st["wg"][m][:, 2 * a:2 * a + 2, :],
                             h2f[:, 2 * a:2 * a + 2, sl],
                             start=(a == 0), stop=(a == DK // 2 - 1),
                             perf_mode=PM_DR)
        for a in range(DK // 2):
            nc.tensor.matmul(hp[:], st["wh"][m][:, 2 * a:2 * a + 2, :],
                             h2f[:, 2 * a:2 * a + 2, sl],
                             start=(a == 0), stop=(a == DK // 2 - 1),
                             perf_mode=PM_DR)
        sg = pfg.tile([128, 256], BF16, name="sg")
        nc.scalar.activation(sg[:], gp[:], AF.Silu, scale=1.0 / 256.0)
        # ghF = sg * (hp/256) * 8  (x8 keeps ghF in fp8 range)
        nc.vector.scalar_tensor_tensor(
            st["ghF"][:, m, sl], hp[:], 1.0 / 32.0, sg[:],
            op0=ALU.mult, op1=ALU.mult)
    pfg.release()
    pf_gps.release()


def _ffn_out_pass(nc, tc, st, x1, out_d, ts_):
    """ghT @ ffn_out + gated residual for two 128-token chunks."""
    bcast = st["bcast"]
    pf = tc.alloc_tile_pool(name=f"pfo{ts_[0]}", bufs=1)
    pf_ops = tc.alloc_tile_pool(name=f"pfo{ts_[0]}_ps", bufs=1, space="PSUM")
    fps = {t: [pf_ops.tile([128, 384], FP32, name=f"fps{t}_{hf}")
               for hf in range(2)] for t in ts_}
    for p in range(MG // 2):
        for t in ts_:
            for hf in range(2):
                nc.tensor.matmul(
                    fps[t][hf][:],
                    st["ghF"][:, 2 * p:2 * p + 2, 128 * t:128 * (t + 1)],
                    st["wo_f"][:, 2 * p:2 * p + 2,
                               384 * hf:384 * (hf + 1)],
                    start=(p == 0), stop=(p == MG // 2 - 1),
                    perf_mode=PM_DR)
    for t in ts_:
        ot = pf.tile([128, D], FP32, name="ot")
        for hf in range(2):
            tt = pf.tile([128, 384], FP32, name="tt")
            # fps carries x(8*32); divide out with the gate multiply
            nc.vector.scalar_tensor_tensor(
                tt[:], fps[t][hf][:], 1.0 / 256.0,
                bcast[5][:, 384 * hf:384 * (hf + 1)],
                op0=ALU.mult, op1=ALU.mult)
            nc.vector.tensor_add(ot[:, 384 * hf:384 * (hf + 1)],
                                 tt[:],
                                 x1[t][:, 384 * hf:384 * (hf + 1)])
        nc.sync.dma_start(out_d.ap()[128 * t:128 * (t + 1), :], ot[:])
    pf.release()
    pf_ops.release()


def _emit(nc, tc, dr, out_d):
    with tc.tile_pool(name="pers", bufs=1) as pers, \
         tc.tile_pool(name="dram", bufs=1, space="DRAM") as dram:
        st = {}
        st["ident_bf"] = pers.tile([128, 128], BF16, name="ident_bf")
        make_identity(nc, st["ident_bf"][:])
        ones_f = pers.tile([1, 128], FP32, name="ones_f")
        nc.vector.memset(ones_f[:], 1.0)
        st["ones_r"] = pers.tile([1, 128], FP32R, name="ones_r")
        nc.scalar.activation(st["ones_r"][:], ones_f[:], AF.Copy)
        st["ones_bf"] = pers.tile([1, 128], BF16, name="ones_bf")
        nc.scalar.activation(st["ones_bf"][:], ones_f[:], AF.Copy)
        onescol_f = pers.tile([128, 1], FP32, name="onescol_f")
        nc.vector.memset(onescol_f[:], 1.0)
        st["onescol_bf"] = pers.tile([128, 1], BF16, name="onescol_bf")
        nc.scalar.activation(st["onescol_bf"][:], onescol_f[:], AF.Copy)
        st["ones3_f"] = pers.tile([128, 3], FP32, name="ones3_f")
        nc.vector.memset(st["ones3_f"][:], 1.0)
        st["eps_sb"] = pers.tile([128, 1], FP32, name="eps_sb")
        nc.vector.memset(st["eps_sb"][:], EPS)

        st["gsel_sb"] = pers.tile([2, 1], BF16, name="gsel_sb")
        nc.sync.dma_start(st["gsel_sb"][:], dr["gsel"].ap())
        st["mod_cols"] = pers.tile([128, 12], FP32, name="mod_cols")
        st["mod_cols_bf"] = pers.tile([128, 12], BF16, name="mod_cols_bf")
        st["bcast"] = {m: pers.tile([128, D], BF16, name=f"bcast{m}")
                       for m in (2, 3, 4, 5)}
        st["sq_j"] = [pers.tile([1, 512], FP32R, name=f"sq_j{j}")
                      for j in range(4)]
        st["rinv_j"] = [pers.tile([1, 512], FP32R, name=f"rinv_j{j}")
                        for j in range(4)]
        st["ones3t"] = pers.tile([65, 64], FP32R, name="ones3t")
        ones3f = pers.tile([65, 64], FP32, name="ones3f")
        nc.vector.memset(ones3f[:], 1.0)
        nc.scalar.activation(st["ones3t"][:], ones3f[:], AF.Copy)
        st["rb"] = pers.tile([128, L], BF16, name="rb")
        st["rb_cols"] = pers.tile([128, 16], FP32, name="rb_cols")
        st["bq_r"] = pers.tile([1, 384], FP32R, name="bq_r")
        st["bvw_r"] = pers.tile([1, HC * HD], FP32R, name="bvw_r")
        st["ar_a_in"] = dram.tile([2, 2 * D], FP32, name="ar_a_in")
        st["ar_a_out"] = dram.tile([2, 2 * D], FP32, name="ar_a_out")
        st["ar_b_in"] = dram.tile([2, 4 * D], FP32, name="ar_b_in")
        st["ar_b_out"] = dram.tile([2, 4 * D], FP32, name="ar_b_out")
        for j in range(4):
            st[f"rs_in{j}"] = dram.tile([512, D], BF16, name=f"rs_in{j}")
            st[f"rs_out{j}"] = dram.tile([QTOK, D], BF16, name=f"rs_out{j}")

        _phase_a1(nc, tc, dr, st)

        with tc.tile_pool(name="p_x1", bufs=1) as p_x1:
            # prefetch residual slice into x1 (rs partial added in later)
            x1 = [p_x1.tile([128, D], BF16, name=f"x1_{t}") for t in range(4)]
            for t in range(4):
                nc.sync.dma_start(
                    x1[t][:], dr["x_slice"].ap()[128 * t:128 * (t + 1), :])

            p_qv = tc.alloc_tile_pool(name="p_qv", bufs=1)
            qkr_A = p_qv.tile([128, L], BF16, name="qkr_A")
            qkr_B = p_qv.tile([128, L], BF16, name="qkr_B")
            qkr_C = p_qv.tile([64, L], BF16, name="qkr_C")
            qkr_D = p_qv.tile([64, L], BF16, name="qkr_D")
            qkr_E = p_qv.tile([128, L], BF16, name="qkr_E")
            qkr_F = p_qv.tile([128, L], BF16, name="qkr_F")
            qkr = (qkr_A, qkr_B, qkr_C, qkr_D, qkr_E, qkr_F)
            v_sb = [p_qv.tile([128, 2, 240], FP8E4, name=f"v_sb{t}")
                    for t in range(KC // 2)]
            wo_sb = [p_qv.tile([64, D], BF16, name=f"wo{h}")
                     for h in range(HC)]
            for h in range(HC):
                nc.sync.dma_start(wo_sb[h][:],
                                  dr["wo"].ap()[64 * h:64 * (h + 1), :])
            wqk_sb = [p_qv.tile([128, 384], BF16, name=f"wqk{k}")
                      for k in range(DK)]
            wv_sb = [p_qv.tile([128, HC * HD], BF16, name=f"wv{k}")
                     for k in range(DK)]
            for k in range(DK):
                nc.sync.dma_start(wqk_sb[k][:],
                                  dr["wqk"].ap()[128 * k:128 * (k + 1), :])
                nc.sync.dma_start(wv_sb[k][:],
                                  dr["wv"].ap()[128 * k:128 * (k + 1), :])

            p_xt = tc.alloc_tile_pool(name="p_xt", bufs=1)
            xt = [p_xt.tile([128, L], BF16, name=f"xt{k}") for k in range(DK)]

            _phase_b_stats(nc, tc, dr, st, xt)
            _phase_a2_cols(nc, tc, dr, st)
            _phase_wmod(nc, tc, st, wqk_sb, wv_sb)
            _phase_c(nc, tc, dr, st, xt, qkr, v_sb, wqk_sb, wv_sb)
            p_xt.release()

            # FFN gate/hidden weights + ghF: loads stream during attention
            pfw = tc.alloc_tile_pool(name="pfw", bufs=1)
            st["wg"] = [pfw.tile([128, DK, 128], FP8E4, name=f"wg{m}")
                        for m in range(MG)]
            st["wh"] = [pfw.tile([128, DK, 128], FP8E4, name=f"wh{m}")
                        for m in range(MG)]
            ghF = pfw.tile([128, MG, TOK], FP8E4, name="ghF")
            st["ghF"] = ghF
            for m in range(MG):
                nc.sync.dma_start(
                    st["wg"][m][:],
                    dr["wg_blk"].ap()[128 * m:128 * (m + 1), :])
                nc.sync.dma_start(
                    st["wh"][m][:],
                    dr["wh_blk"].ap()[128 * m:128 * (m + 1), :])
            p_h2t = tc.alloc_tile_pool(name="p_h2t", bufs=1)
            h2t = p_h2t.tile([128, DK, TOK], FP8E4, name="h2f")

            pwo = tc.alloc_tile_pool(name="pwo", bufs=1)
            st["wo_f"] = pwo.tile([128, MG, D], FP8E4, name="wof")
            nc.sync.dma_start(st["wo_f"][:], dr["wout2"].ap())

            _phase_a2_bcast(nc, tc, dr, st)

            pfp0 = tc.alloc_tile_pool(name="pfp0", bufs=1)
            prep0, h2cs0 = _ffn_prep_closures(nc, tc, st, x1, 0, pfp0)

            def out1_dma(j, tc4, out1):
                nc.sync.dma_start(
                    st[f"rs_in{j}"][128 * tc4:128 * (tc4 + 1), :], out1[:])

            def fillers_for(p, trps):
                if p == 4:
                    return prep0
                if p == 5:
                    return _ffn_tr_closures(nc, tc, st, h2t, 0, h2cs0, trps)
                return []

            _attention_core(nc, tc, st, qkr, v_sb, wo_sb, out1_dma,
                            fillers_for)
            pfp0.release()

            _ffn_gatehid(nc, tc, st, h2t, 0)
            _ffn_out_pass(nc, tc, st, x1, out_d, (0, 1))

            pfp1 = tc.alloc_tile_pool(name="pfp1", bufs=1)
            prep1, h2cs1 = _ffn_prep_closures(nc, tc, st, x1, 1, pfp1)
            for f in prep1:
                f()
            with tc.tile_pool(name="trp1_ps", bufs=2, space="PSUM") as trps1:
                for f in _ffn_tr_closures(nc, tc, st, h2t, 1, h2cs1, trps1):
                    f()
            pfp1.release()
            _ffn_gatehid(nc, tc, st, h2t, 1)
            _ffn_out_pass(nc, tc, st, x1, out_d, (2, 3))
            pwo.release()
            p_h2t.release()
            pfw.release()
            p_qv.release()


def _attention_core(nc, tc, st, qkr, v_sb, wo_sb, out1_dma, fillers_for):
    """Head-paired attention: 6 pairs of (j,h) streams, one at array rows
    0-63 and one at rows 64-127, whose score matmuls run concurrently as
    row tiles into the two banks of a shared wide PSUM tile. One wide exp
    [128,1024] covers both streams' chunk; PV runs as fp8 DoubleRow over
    chunk pairs. Per-quarter ReduceScatter after each j completes."""
    bcast = st["bcast"]
    qkr_A, qkr_B, qkr_C, qkr_D, qkr_E, qkr_F = qkr

    def kq(j, h, base):
        jsl = slice(512 * j, 512 * (j + 1))
        if h == 0:
            return (lambda s: qkr_B[0:64, s]), qkr_A[0:64, jsl]
        if h == 1:
            return (lambda s: qkr_B[64:128, s]), qkr_A[64:128, jsl]
        if base == 0:
            return (lambda s: qkr_D[0:64, s]), qkr_C[0:64, jsl]
        return (lambda s: qkr_F[64:128, s]), qkr_E[64:128, jsl]

    # (j, h, array-base) stream pairs; second element always at rows 64+
    PAIRS = [((0, 0, 0), (0, 1, 64)), ((0, 2, 0), (1, 1, 64)),
             ((1, 0, 0), (1, 2, 64)), ((2, 0, 0), (2, 1, 64)),
             ((2, 2, 0), (3, 1, 64)), ((3, 0, 0), (3, 2, 64))]
    # pair index after which each j is fully computed
    J_DONE = {1: 0, 2: 1, 4: 2, 5: 3}

    with tc.tile_pool(name="at", bufs=2) as at, \
         tc.tile_pool(name="at_exp", bufs=3) as at_exp, \
         tc.tile_pool(name="at_sps", bufs=2, space="PSUM") as at_sps, \
         tc.tile_pool(name="at_ops", bufs=1, space="PSUM") as at_ops, \
         tc.tile_pool(name="at_shps", bufs=2, space="PSUM") as at_shps:
        dens = {j: at.tile([65, 512], FP32, name=f"den{j}", bufs=1)
                for j in range(JT)}
        o65 = {}
        o_sb = {}

        def emit_jtail(j):
            den_r = at.tile([65, 512], FP32R, name="den_r")
            nc.vector.reciprocal(den_r[:], dens[j][:])
            for h in range(HC):
                bps = at_shps.tile([128, 512], FP32, name="bps", tag="sh")
                nc.tensor.matmul(bps[0:64, :],
                                 st["ones3t"][32 * h:32 * h + 1, :],
                                 den_r[32 * h:32 * h + 1, :],
                                 start=True, stop=True)
                rb64 = at.tile([64, 512], BF16, name="rb64")
                nc.vector.tensor_copy(rb64[:], bps[0:64, :])
                osb = at.tile([64, 512], BF16, name=f"osb_{j}_{h}", bufs=1)
                nc.vector.tensor_mul(osb[:], o65[(j, h)][:], rb64[:])
                o_sb[(j, h)] = osb
            for tc4 in range(4):
                out1 = at.tile([128, D], BF16, name="out1", bufs=2)
                for half in range(2):
                    wps = at_shps.tile([128, 384], FP32, name="wps", tag="sh")
                    for h in range(HC):
                        nc.tensor.matmul(
                            wps[:],
                            o_sb[(j, h)][:, 128 * tc4:128 * (tc4 + 1)],
                            wo_sb[h][:, 384 * half:384 * (half + 1)],
                            start=(h == 0), stop=(h == HC - 1))
                    nc.vector.tensor_mul(
                        out1[:, 384 * half:384 * (half + 1)], wps[:],
                        bcast[2][:, 384 * half:384 * (half + 1)])
                out1_dma(j, tc4, out1)
            nc.gpsimd.collective_compute(
                "ReduceScatter", ALU.add, replica_groups=GROUPS,
                ins=[st[f"rs_in{j}"].opt()], outs=[st[f"rs_out{j}"].opt()])

        for p, (sA, sB) in enumerate(PAIRS):
            fiter = iter(fillers_for(p, at_shps))
            jA, hA, _ = sA
            jB, hB, _ = sB
            kA, qA = kq(*sA)
            kB, qB = kq(*sB)
            opsA = at_ops.tile([128, 512], FP32, name="opsA")
            opsB = at_ops.tile([128, 512], FP32, name="opsB")
            exs = {}
            for dg in range(9):
                if dg < 8:
                    ex2 = at_exp.tile([128, 2, 1024], FP8E4, name="ex2")
                    for c in range(2):
                        kc = 2 * dg + c
                        ksl = slice(128 * kc, 128 * (kc + 1))
                        wide = at_sps.tile([128, 1024], FP32, name="wide")
                        nc.tensor.matmul(wide[:, 0:512], kA(ksl), qA,
                                         start=True, stop=True)
                        nc.tensor.matmul(wide[:, 512:1024], kB(ksl), qB,
                                         start=True, stop=True)
                        nc.scalar.activation(ex2[:, c, :], wide[:], AF.Exp,
                                             scale=1.0 / SCALE)
                    exs[dg] = ex2
                if dg >= 1:
                    gg = dg - 1
                    ex2 = exs.pop(gg)
                    nc.tensor.matmul(
                        opsA[0:65, :], v_sb[gg][:, :, 80 * hA:80 * hA + 65],
                        ex2[:, :, 0:512],
                        start=(gg == 0), stop=(gg == 7), perf_mode=PM_DR)
                    nc.tensor.matmul(
                        opsB[0:65, :], v_sb[gg][:, :, 80 * hB:80 * hB + 65],
                        ex2[:, :, 512:1024],
                        start=(gg == 0), stop=(gg == 7), perf_mode=PM_DR)
                    for _ in range(2):
                        f = next(fiter, None)
                        if f is None:
                            break
                        f()
            for (jx, hx, ops) in ((jA, hA, opsA), (jB, hB, opsB)):
                o = at.tile([64, 512], BF16, name=f"o65_{jx}_{hx}", bufs=1)
                nc.vector.tensor_copy(o[:], ops[0:64, :])
                o65[(jx, hx)] = o
                nc.vector.tensor_copy(dens[jx][32 * hx:32 * hx + 1, :],
                                      ops[64:65, :])
            if p in J_DONE:
                emit_jtail(J_DONE[p])
            for f in fiter:
                f()


# ---------------------------------------------------------------- entry
def get_program(reps=1):
    key = f"nc{reps}"
    if key not in _CACHE:
        _CACHE[key] = build_program(reps)
    return _CACHE[key]


def make_in_maps(inputs):
    cores = host_prep(inputs)
    names = [s[0] for s in DRAM_SPECS]
    return [{n: cores[i][n] for n in names} for i in range(NC_)]


def kernel(**inputs):
    nc = get_program()
    in_maps = make_in_maps(inputs)
    res = bass_utils.run_bass_kernel_spmd(nc, in_maps, list(range(NC_)))
    out = np.zeros((B, L, D), np.float32)
    for i in range(NC_):
        g, r = i // G, i % G
        o = res.results[i]["out"]
        for t in range(4):
            out[g, 512 * t + QTOK * r: 512 * t + QTOK * (r + 1)] = \
                o[QTOK * t:QTOK * (t + 1)]
    return out


# revision 20
# speedup vs baseline: 1.1458x; 1.1458x over previous
"""DiT block kernel for 8 TRN2 NeuronCores (self-contained).

Sharding: cores 0-3 <-> batch 0, cores 4-7 <-> batch 1.
Per 4-core group: attention head-parallel (3 of 12 heads/core, all 2048
tokens), W_o row-sharded -> per-quarter ReduceScatter (4x bf16) -> each
core owns 4x128-token slices; FFN token-parallel (512 rows, fp8 weights
prefetched to SBUF during attention). AdaLN/cond path is DH-sharded over
all 8 cores with host-folded (cond_w2 @ W_mod) matrices -> two small
AllReduces overlapped with the rms-stats phase (a dummy AllReduce at t=0
absorbs the collective entry barrier).

The attention-norm modulation is folded into the QKV weights on-chip:
wqk/wv rows are scaled by gamma after the AllReduce lands, the beta term
is injected as a rank-1 matmul (beta^T W (x) rms), and the 1/rms factor
is applied to the matmul outputs (rows for q/k via a broadcast tile,
columns for v via per-partition scalars). This removes the full-width
modulated-h pass. Attention runs with 1024-wide double-buffered score
groups: 2 score matmuls -> one wide exp -> 2 PV matmuls, so the PE
stream never blocks on the ScalarE exp. Softmax denominators use the
ones-augmented V trick; reciprocals are batched [3,512] per q-tile.
"""
import numpy as np
import ml_dtypes

import concourse.bass as bass
import concourse.mybir as mybir
import concourse.tile as tile
from concourse import bacc, bass_utils
from concourse.masks import make_identity

FP32 = mybir.dt.float32
FP32R = mybir.dt.float32r
BF16 = mybir.dt.bfloat16
FP8E4 = mybir.dt.float8e4
PM_DR = mybir.MatmulPerfMode.DoubleRow
AF = mybir.ActivationFunctionType
ALU = mybir.AluOpType
AX = mybir.AxisListType

NPBF = ml_dtypes.bfloat16

B, L, D, H, DH = 2, 2048, 768, 12, 3072
HD = 64
EPS = 1e-6
SCALE = float(np.sqrt(HD))
NC_ = 8
G = 4            # cores per batch group
HC = 3           # heads per core
TOK = L // G     # 512
QTOK = 128       # tokens per core per quarter
DH8 = DH // NC_  # 384
GROUPS = [[0, 1, 2, 3], [4, 5, 6, 7]]
KC = L // 128    # 16 key chunks
JT = L // 512    # 4 q tiles
DK = D // 128    # 6 d chunks
MG = DH // 128   # 24 dh chunks


def _bf(a):
    return np.ascontiguousarray(np.asarray(a, np.float32)).astype(NPBF)


NPF8 = ml_dtypes.float8_e4m3


def _f8(a, scale):
    a = np.asarray(a, np.float32) * scale
    return np.ascontiguousarray(np.clip(a, -240.0, 240.0)).astype(NPF8)


# ---------------------------------------------------------------- host prep
def host_prep(inp):
    f = {k: np.ascontiguousarray(np.asarray(v, np.float32)) for k, v in inp.items()}
    x, c = f["x"], f["c"]
    cos, sin = f["freqs_cos"], f["freqs_sin"]          # [L, 32]

    attn_gamma_s = f["attn_gamma"] * f["attn_norm_w"][None, :]
    ffn_gamma_s = f["ffn_gamma"] * f["ffn_norm_w"][None, :]
    mods = [attn_gamma_s, f["attn_beta"], f["attn_alpha"],
            ffn_gamma_s, f["ffn_beta"], f["ffn_gamma"]]
    wfold_full = [f["cond_w2"] @ m for m in mods]       # [DH, D] x6
    bvec = np.stack([f["cond_b2"] @ m for m in mods])   # [6, D]
    # FFN norm/modulation path carries an extra x8 so h2 lands in fp8
    # e4m3's normal range; gate/hidden weights carry x32. The product
    # 8*32=256 is divided back out after the gate/hidden matmuls.
    for mi in (3, 4):
        wfold_full[mi] = wfold_full[mi] * 8.0
        bvec[mi] = bvec[mi] * 8.0

    perm = np.concatenate([np.arange(0, HD, 2), np.arange(1, HD, 2)])
    cosT, sinT = cos.T, sin.T                            # [32, L]
    cct = np.tile(cosT, (4, 1)).astype(np.float32)       # [128, L]
    sst = np.concatenate([-sinT, sinT, -sinT, sinT], 0).astype(np.float32)

    cT = np.ascontiguousarray(c.T)                       # [768, 2]
    ct_pack = cT.reshape(6, 128, 2).transpose(1, 0, 2).reshape(128, 12).copy()

    wg_blk = f["ffn_gate"].reshape(6, 128, 24, 128).transpose(2, 1, 0, 3) \
        .reshape(24 * 128, 768)
    wh_blk = f["ffn_hidden"].reshape(6, 128, 24, 128).transpose(2, 1, 0, 3) \
        .reshape(24 * 128, 768)
    # [dh-in-chunk 128, dh-chunk 24, d 768] for DoubleRow rhs slices
    wout2 = f["ffn_out"].reshape(24, 128, 768).transpose(1, 0, 2) \
        .reshape(128, 24 * 768)

    pswap = np.zeros((128, 128), np.float32)
    for i_ in range(128):
        pswap[i_, i_ ^ 32] = 1.0

    cores = []
    for i in range(NC_):
        g, r = i // G, i % G
        hs = [HC * r + j for j in range(HC)]
        si = slice(DH8 * i, DH8 * (i + 1))
        blocks = [f["W_q"][:, h * HD:(h + 1) * HD][:, perm] for h in hs]
        blocks += [f["W_k"][:, h * HD:(h + 1) * HD][:, perm] for h in hs]
        wqk = np.concatenate(blocks, 1)                  # [768, 384]
        wv = np.concatenate(
            [f["W_v"][:, h * HD:(h + 1) * HD] for h in hs], 1)  # [768, 192]
        wo = np.concatenate([f["W_o"][h * HD:(h + 1) * HD] for h in hs], 0)

        # token ownership: 4 slices of 128, one per query quarter
        x_slice = np.concatenate(
            [x[g, 512 * t + QTOK * r: 512 * t + QTOK * (r + 1)]
             for t in range(4)], 0)                      # [512, 768]

        gsel = np.zeros((2, 1), np.float32)
        gsel[g, 0] = 1.0

        cores.append(dict(
            xT=_bf(x[g].T),
            x_slice=_bf(x_slice),
            cct=_bf(cct), sst=_bf(sst),
            ct_pack=_bf(ct_pack),
            w1s=_bf(f["cond_w1"][:, si]),
            b1_rows=np.ascontiguousarray(
                np.tile(f["cond_b1"][si][None, :], (2, 1))),    # [2, 384]
            wfold=_bf(np.concatenate([w[si] for w in wfold_full], 0)),
            gsel=_bf(gsel),
            bv_cat=np.ascontiguousarray(bvec.reshape(1, 6 * D)),
            pswap=_bf(pswap),
            wqk=_bf(wqk), wv=_bf(wv), wo=_bf(wo),
            wg_blk=_f8(wg_blk, 32.0), wh_blk=_f8(wh_blk, 32.0),
            wout2=_f8(wout2, 32.0),
        ))
    return cores


# ---------------------------------------------------------------- program
_CACHE = {}

DRAM_SPECS = [
    ("xT", [D, L], BF16),
    ("x_slice", [TOK, D], BF16),
    ("cct", [128, L], BF16),
    ("sst", [128, L], BF16),
    ("ct_pack", [128, 12], BF16),
    ("w1s", [D, DH8], BF16),
    ("b1_rows", [2, DH8], FP32),
    ("wfold", [6 * DH8, D], BF16),
    ("gsel", [2, 1], BF16),
    ("bv_cat", [1, 6 * D], FP32),
    ("pswap", [128, 128], BF16),
    ("wqk", [D, 384], BF16),
    ("wv", [D, HC * HD], BF16),
    ("wo", [HC * HD, D], BF16),
    ("wg_blk", [MG * 128, D], FP8E4),
    ("wh_blk", [MG * 128, D], FP8E4),
    ("wout2", [128, MG * D], FP8E4),
]


def build_program(reps=1):
    nc = bacc.Bacc("TRN2", target_bir_lowering=False, debug=False,
                   num_devices=NC_)
    dr = {}
    for name, shape, dt in DRAM_SPECS:
        dr[name] = nc.dram_tensor(name, shape, dt, kind="ExternalInput")
    out_d = nc.dram_tensor("out", [TOK, D], FP32, kind="ExternalOutput")

    with tile.TileContext(nc) as tc, \
         nc.allow_low_precision(reason="bf16 matmuls, fp32 PSUM accumulation"):
        for _ in range(reps):
            _emit(nc, tc, dr, out_d)
    nc.compile()
    return nc


def _phase_a1(nc, tc, dr, st):
    """cond MLP partials (row layout); AR for mods 0,1 issued early."""
    with tc.tile_pool(name="pa", bufs=1) as pa, \
         tc.tile_pool(name="pa_wf", bufs=6) as pa_wf, \
         tc.tile_pool(name="pa_ps", bufs=2, space="PSUM") as pa_ps:
        ct_sb = pa.tile([128, 12], BF16, name="ct_sb")
        nc.sync.dma_start(ct_sb[:], dr["ct_pack"].ap())
        b1_sb = pa.tile([2, DH8], FP32, name="b1_sb")
        nc.sync.dma_start(b1_sb[:], dr["b1_rows"].ap())
        w1_sb = [pa.tile([128, DH8], BF16, name=f"w1_sb{k}")
                 for k in range(DK)]
        for k in range(DK):
            nc.sync.dma_start(w1_sb[k][:],
                              dr["w1s"].ap()[128 * k:128 * (k + 1), :])

        # h1 [2, 384] = c @ W1s  (both batches)
        h1_ps = pa_ps.tile([2, DH8], FP32, name="h1_ps")
        for k in range(DK):
            nc.tensor.matmul(h1_ps[:], ct_sb[:, 2 * k:2 * k + 2],
                             w1_sb[k][:],
                             start=(k == 0), stop=(k == DK - 1))
        h1_sb = pa.tile([2, DH8], FP32, name="h1_sb")
        nc.vector.tensor_add(h1_sb[:], h1_ps[:], b1_sb[:])
        silu_r = pa.tile([2, DH8], BF16, name="silu_r")
        nc.scalar.activation(silu_r[:], h1_sb[:], AF.Silu)
        # silu columns [128, 2] x3 chunks
        silu_c = pa.tile([128, 6], BF16, name="silu_c")
        scp = pa_ps.tile([128, 6], BF16, name="scp")
        for cch in range(3):
            nc.tensor.transpose(scp[:, 2 * cch:2 * cch + 2],
                                silu_r[:, 128 * cch:128 * (cch + 1)],
                                st["ident_bf"][0:2, 0:2])
        nc.vector.tensor_copy(silu_c[:], scp[:])

        arin_a = pa.tile([2, 2 * D], FP32, name="arin_a")
        arin_b = pa.tile([2, 4 * D], FP32, name="arin_b")
        for m in range(6):
            for half in range(2):
                mp = pa_ps.tile([2, 384], FP32, name="mp")
                for k in range(3):
                    wf = pa_wf.tile([128, D], BF16, name="wf")
                    if half == 0:
                        nc.sync.dma_start(
                            wf[:], dr["wfold"].ap()[128 * (3 * m + k):
                                                    128 * (3 * m + k + 1), :])
                        st[f"wf{m}_{k}"] = wf
                    else:
                        wf = st[f"wf{m}_{k}"]
                    nc.tensor.matmul(mp[:],
                                     silu_c[:, 2 * k:2 * k + 2],
                                     wf[:, 384 * half:384 * (half + 1)],
                                     start=(k == 0), stop=(k == 2))
                if m < 2:
                    dst = arin_a[0:2, m * D + 384 * half:
                                 m * D + 384 * (half + 1)]
                else:
                    dst = arin_b[0:2, (m - 2) * D + 384 * half:
                                 (m - 2) * D + 384 * (half + 1)]
                nc.vector.tensor_copy(dst, mp[:])
            if m == 1:
                nc.sync.dma_start(st["ar_a_in"][:], arin_a[:])
                nc.gpsimd.collective_compute(
                    "AllReduce", ALU.add,
                    replica_groups=[list(range(NC_))],
                    ins=[st["ar_a_in"].opt()], outs=[st["ar_a_out"].opt()])
        nc.sync.dma_start(st["ar_b_in"][:], arin_b[:])
        nc.gpsimd.collective_compute(
            "AllReduce", ALU.add, replica_groups=[list(range(NC_))],
            ins=[st["ar_b_in"].opt()], outs=[st["ar_b_out"].opt()])


def _mod_row(nc, st, pa2_ps, ar_bf, bv_sb, mod_bf, base_col, m):
    """One mod row [1, D] at partition 0 from the reduced tensor."""
    for half in range(2):
        sl = slice(384 * half, 384 * (half + 1))
        sp = pa2_ps.tile([1, 384], FP32, name="sp")
        nc.tensor.matmul(sp[:], st["gsel_sb"][:],
                         ar_bf[0:2, base_col + 384 * half:
                               base_col + 384 * (half + 1)])
        nc.vector.tensor_add(mod_bf[0:1, sl], sp[:],
                             bv_sb[0:1, m * D + 384 * half:
                                   m * D + 384 * (half + 1)])


def _phase_a2_cols(nc, tc, dr, st):
    """Consume AR-a -> mod_cols (gamma/beta for attention norm)."""
    with tc.tile_pool(name="pa2c", bufs=1) as pa2, \
         tc.tile_pool(name="pa2c_ps", bufs=2, space="PSUM") as pa2_ps:
        ar_sb = pa2.tile([2, 2 * D], FP32, name="ara_sb")
        nc.sync.dma_start(ar_sb[:], st["ar_a_out"][:])
        ar_bf = pa2.tile([2, 2 * D], BF16, name="ara_bf")
        nc.vector.tensor_copy(ar_bf[:], ar_sb[:])
        bv_sb = pa2.tile([1, 2 * D], FP32, name="bv_a")
        nc.sync.dma_start(bv_sb[:], dr["bv_cat"].ap()[0:1, 0:2 * D])
        mod_bf = [pa2.tile([1, D], BF16, name=f"mod_ab{m}") for m in range(2)]
        for m in range(2):
            _mod_row(nc, st, pa2_ps, ar_bf, bv_sb, mod_bf[m], m * D, m)
        # even psum cols so each bf16 write stays 4-byte aligned
        mcp = pa2_ps.tile([128, 24], BF16, name="mcp")
        for m in range(2):
            for k in range(DK):
                col = 2 * (6 * m + k)
                nc.tensor.transpose(mcp[:, col:col + 1],
                                    mod_bf[m][0:1, 128 * k:128 * (k + 1)],
                                    st["ident_bf"][0:1, 0:1])
        nc.vector.tensor_copy(st["mod_cols"][:], mcp[:, 0:24:2])
        nc.vector.tensor_copy(st["mod_cols_bf"][:], mcp[:, 0:24:2])


def _phase_a2_bcast(nc, tc, dr, st):
    """Consume AR-b -> bcast tiles for mods 2..5."""
    bcast = st["bcast"]
    with tc.tile_pool(name="pa2b", bufs=1) as pa2, \
         tc.tile_pool(name="pa2b_ps", bufs=2, space="PSUM") as pa2_ps:
        ar_sb = pa2.tile([2, 4 * D], FP32, name="arb_sb")
        nc.sync.dma_start(ar_sb[:], st["ar_b_out"][:])
        ar_bf = pa2.tile([2, 4 * D], BF16, name="arb_bf")
        nc.vector.tensor_copy(ar_bf[:], ar_sb[:])
        bv_sb = pa2.tile([1, 6 * D], FP32, name="bv_b")
        nc.sync.dma_start(bv_sb[:], dr["bv_cat"].ap())
        for m in (2, 3, 4, 5):
            mod_bf = pa2.tile([1, D], BF16, name=f"mod_bb{m}")
            _mod_row(nc, st, pa2_ps, ar_bf, bv_sb, mod_bf, (m - 2) * D, m)
            for half in range(2):
                sl = slice(384 * half, 384 * (half + 1))
                bp = pa2_ps.tile([128, 384], FP32, name="bp")
                nc.tensor.matmul(bp[:], st["ones_bf"][:],
                                 mod_bf[0:1, sl])
                nc.vector.tensor_copy(bcast[m][:, sl], bp[:])


def _phase_b_stats(nc, tc, dr, st, xt):
    """xT load + rms stats -> rms rows (sq_r), 1/rms rows (rinv_r),
    broadcast 1/rms tile rb [128,L], and column-form rb_cols [128,16]."""
    with tc.tile_pool(name="pb", bufs=2) as pb, \
         tc.tile_pool(name="pb_ps", bufs=1, space="PSUM") as pb_ps:
        for k in range(DK):
            nc.sync.dma_start(xt[k][:],
                              dr["xT"].ap()[128 * k:128 * (k + 1), :])
        msq = [pb_ps.tile([1, 512], FP32, name=f"msq{j}") for j in range(4)]
        for k in range(DK):
            xsq = pb.tile([128, L], BF16, name="xsq")
            nc.scalar.activation(xsq[:], xt[k][:], AF.Square)
            for j in range(4):
                nc.tensor.matmul(msq[j][:], st["onescol_bf"][:],
                                 xsq[:, 512 * j:512 * (j + 1)],
                                 start=(k == 0), stop=(k == DK - 1))
        sqb = [pb.tile([1, 512], BF16, name=f"sqb{j}") for j in range(4)]
        for j in range(4):
            nc.scalar.activation(st["sq_j"][j][:], msq[j][:], AF.Sqrt,
                                 bias=st["eps_sb"][0:1, 0:1], scale=1.0 / D)
            nc.scalar.activation(sqb[j][:], msq[j][:], AF.Sqrt,
                                 bias=st["eps_sb"][0:1, 0:1], scale=1.0 / D)
        # rms rows -> columns (transpose), one cheap per-partition
        # reciprocal, then back to rows for the broadcast tile
        scp = pb_ps.tile([128, 32], BF16, name="scp2")
        for t in range(KC):
            nc.tensor.transpose(scp[:, 2 * t:2 * t + 1],
                                sqb[t // 4][0:1,
                                            128 * (t % 4):128 * (t % 4) + 128],
                                st["ident_bf"][0:1, 0:1])
        sq_cols = pb.tile([128, 16], BF16, name="sq_cols")
        nc.vector.tensor_copy(sq_cols[:], scp[:, 0:32:2])
        nc.vector.reciprocal(st["rb_cols"][:], sq_cols[:])
        rbc_bf = pb.tile([128, 16], BF16, name="rbc_bf")
        nc.vector.tensor_copy(rbc_bf[:], st["rb_cols"][:])
        for j in range(4):
            rrp = pb_ps.tile([1, 512], BF16, name="rrp", bufs=1)
            for tq in range(4):
                nc.tensor.transpose(rrp[0:1, 128 * tq:128 * (tq + 1)],
                                    rbc_bf[:, 4 * j + tq:4 * j + tq + 1],
                                    st["ident_bf"][:])
            nc.vector.tensor_copy(st["rinv_j"][j][:], rrp[:])
            rbp = pb_ps.tile([128, 512], FP32, name="rbp", bufs=1)
            nc.tensor.matmul(rbp[:], st["ones_bf"][:], st["rinv_j"][j][:])
            nc.vector.tensor_copy(st["rb"][:, 512 * j:512 * (j + 1)], rbp[:])


def _phase_wmod(nc, tc, st, wqk_sb, wv_sb):
    """beta^T W rows (rank-1 bias operands) + in-place gamma row-scaling
    of the QKV weight tiles. Runs right after AR-a lands."""
    with tc.tile_pool(name="pwm_ps", bufs=2, space="PSUM") as pwm_ps:
        bq_ps = pwm_ps.tile([1, 384], FP32, name="bq_ps")
        for k in range(DK):
            nc.tensor.matmul(bq_ps[:], st["mod_cols_bf"][:, 6 + k:7 + k],
                             wqk_sb[k][:],
                             start=(k == 0), stop=(k == DK - 1))
        nc.scalar.activation(st["bq_r"][:], bq_ps[:], AF.Copy)
        bv_ps = pwm_ps.tile([1, HC * HD], FP32, name="bv_ps")
        for k in range(DK):
            nc.tensor.matmul(bv_ps[:], st["mod_cols_bf"][:, 6 + k:7 + k],
                             wv_sb[k][:],
                             start=(k == 0), stop=(k == DK - 1))
        nc.scalar.activation(st["bvw_r"][:], bv_ps[:], AF.Copy)
        for k in range(DK):
            nc.vector.tensor_scalar(wqk_sb[k][:], wqk_sb[k][:],
                                    st["mod_cols"][:, k:k + 1], None,
                                    op0=ALU.mult)
            nc.vector.tensor_scalar(wv_sb[k][:], wv_sb[k][:],
                                    st["mod_cols"][:, k:k + 1], None,
                                    op0=ALU.mult)


def _phase_c(nc, tc, dr, st, xt, qkr, v_sb, wqk_sb, wv_sb):
    """QKV matmuls on raw xT with folded modulation + RoPE + v tiles."""
    with tc.tile_pool(name="pc_w", bufs=1) as pc_w, \
         tc.tile_pool(name="pc", bufs=2) as pc, \
         tc.tile_pool(name="pc_ps", bufs=2, space="PSUM") as pc_ps:
        cct_sb = pc_w.tile([128, L], BF16, name="cct_sb")
        sst_sb = pc_w.tile([128, L], BF16, name="sst_sb")
        nc.sync.dma_start(cct_sb[:], dr["cct"].ap())
        nc.sync.dma_start(sst_sb[:], dr["sst"].ap())
        pswap_sb = pc_w.tile([128, 128], BF16, name="pswap_sb")
        nc.sync.dma_start(pswap_sb[:], dr["pswap"].ap())

        for m in range(3):
            qk_sb_m = pc.tile([128, L], BF16, name="qk_sb", bufs=2)
            for j in range(4):
                qkp = pc_ps.tile([128, 512], FP32, name="qkp")
                # rank-1: (Wq^T beta) (x) rms; the later 1/rms multiply
                # turns it into the plain beta bias term.
                nc.tensor.matmul(qkp[:], st["bq_r"][0:1, 128 * m:128 * (m + 1)],
                                 st["sq_j"][j][:],
                                 start=True, stop=False)
                for k in range(DK):
                    nc.tensor.matmul(qkp[:],
                                     wqk_sb[k][:, 128 * m:128 * (m + 1)],
                                     xt[k][:, 512 * j:512 * (j + 1)],
                                     start=False, stop=(k == DK - 1))
                nc.vector.tensor_mul(qk_sb_m[:, 512 * j:512 * (j + 1)],
                                     qkp[:], st["rb"][:, 512 * j:512 * (j + 1)])
            t1 = pc.tile([128, L], BF16, name="t1", bufs=1)
            t2 = pc.tile([128, L], BF16, name="t2", bufs=1)
            nc.vector.tensor_mul(t1[:], qk_sb_m[:], cct_sb[:])
            for j in range(4):
                sl = slice(512 * j, 512 * (j + 1))
                swp = pc_ps.tile([128, 512], FP32, name="swp")
                nc.tensor.matmul(swp[:], pswap_sb[:], qk_sb_m[:, sl])
                nc.vector.tensor_mul(t2[:, sl], swp[:], sst_sb[:, sl])
            qkr_A, qkr_B, qkr_C, qkr_D, qkr_E, qkr_F = qkr
            if m == 0:      # (q0, q1) -> A full
                nc.vector.tensor_add(qkr_A[:], t1[:], t2[:])
            elif m == 1:    # (q2, k0) -> C[0:64], B[0:64]; q2 dup -> E[64:]
                nc.vector.tensor_add(qkr_C[:], t1[0:64, :], t2[0:64, :])
                nc.vector.tensor_add(qkr_E[64:128, :], t1[0:64, :],
                                     t2[0:64, :])
                nc.vector.tensor_add(qkr_B[0:64, :], t1[64:128, :],
                                     t2[64:128, :])
            else:           # (k1, k2) -> B[64:128], D[0:64]; k2 dup -> F[64:]
                nc.vector.tensor_add(qkr_B[64:128, :], t1[0:64, :],
                                     t2[0:64, :])
                nc.vector.tensor_add(qkr_D[:], t1[64:128, :],
                                     t2[64:128, :])
                nc.vector.tensor_add(qkr_F[64:128, :], t1[64:128, :],
                                     t2[64:128, :])

        for t in range(KC):
            vp = pc_ps.tile([128, HC * HD], FP32, name="vp")
            # rank-1 beta term (x) rms rows; divided out by rb_cols below
            nc.tensor.matmul(
                vp[:],
                st["sq_j"][t // 4][0:1, 128 * (t % 4):128 * (t % 4) + 128],
                st["bvw_r"][:], start=True, stop=False)
            for k in range(DK):
                nc.tensor.matmul(vp[:], xt[k][:, 128 * t:128 * (t + 1)],
                                 wv_sb[k][:],
                                 start=False, stop=(k == DK - 1))
            for h in range(HC):
                nc.vector.tensor_scalar(
                    v_sb[t // 2][:, t % 2, 80 * h:80 * h + 64],
                    vp[:, 64 * h:64 * (h + 1)],
                    st["rb_cols"][:, t:t + 1], None, op0=ALU.mult)
            nc.scalar.activation(v_sb[t // 2][:, t % 2, 64:225:80],
                                 st["ones3_f"][:], AF.Copy)


def _ffn_prep_closures(nc, tc, st, x1, half, pf):
    """Residual add + rms stats + modulation for a 256-token half, on DVE.
    Returns (closures, h2cs dict filled as closures run)."""
    bcast = st["bcast"]
    ts_ = (0, 1) if half == 0 else (2, 3)
    h2cs = {}
    ms = pf.tile([128, 2], FP32, name=f"ms{half}")
    rv = pf.tile([128, 2], FP32, name=f"rv{half}")

    def stat(idx, t):
        def run():
            rsx = pf.tile([128, D], BF16, name="rsx")
            nc.sync.dma_start(rsx[:], st[f"rs_out{t}"][:])
            nc.vector.tensor_add(x1[t][:], x1[t][:], rsx[:])
            sq = pf.tile([128, D], BF16, name="sq")
            nc.vector.tensor_mul(sq[:], x1[t][:], x1[t][:])
            nc.vector.reduce_sum(ms[:, idx:idx + 1], sq[:], axis=AX.X)
        return run

    def rsq():
        sr = pf.tile([128, 2], FP32, name=f"sr{half}")
        nc.scalar.activation(sr[:], ms[:], AF.Sqrt,
                             bias=st["eps_sb"][:, 0:1], scale=1.0 / D)
        nc.vector.reciprocal(rv[:], sr[:])

    def modt(idx, t):
        def run():
            h2a = pf.tile([128, D], BF16, name="h2a")
            nc.vector.tensor_scalar(h2a[:], x1[t][:], rv[:, idx:idx + 1],
                                    None, op0=ALU.mult)
            h2m = pf.tile([128, D], BF16, name="h2m")
            nc.vector.tensor_mul(h2m[:], h2a[:], bcast[3][:])
            h2c = pf.tile([128, D], BF16, name=f"h2c{t}")
            nc.vector.tensor_add(h2c[:], h2m[:], bcast[4][:])
            h2cs[t] = h2c
        return run

    closures = [stat(0, ts_[0]), stat(1, ts_[1]), rsq,
                modt(0, ts_[0]), modt(1, ts_[1])]
    return closures, h2cs


def _ffn_tr_closures(nc, tc, st, h2f, half, h2cs, trps):
    """Transpose h2c -> h2f (fp8, plane-blocked); 4 closures of 3 chunks."""
    ident = st["ident_bf"]
    ts_ = (0, 1) if half == 0 else (2, 3)
    out = []
    for t in ts_:
        for half_k in range(2):
            ks = range(3 * half_k, 3 * half_k + 3)

            def run(t=t, ks=ks):
                for k in ks:
                    tp = trps.tile([128, 128], BF16, name="trp", tag="sh")
                    nc.tensor.transpose(tp[:],
                                        h2cs[t][:, 128 * k:128 * (k + 1)],
                                        ident[:])
                    nc.vector.tensor_copy(h2f[:, k, 128 * t:128 * (t + 1)],
                                          tp[:])
            out.append(run)
    return out


def _ffn_gatehid(nc, tc, st, h2f, half):
    """Gate/hidden fp8 DoubleRow matmuls + SwiGLU for one token half.
    Weights carry x32, h2 carries x8 -> psum is 256x; divided out in
    the silu scale and the ghF write (which re-applies x8 for fp8)."""
    sl = slice(256 * half, 256 * (half + 1))
    pfg = tc.alloc_tile_pool(name=f"pfg{half}", bufs=2)
    pf_gps = tc.alloc_tile_pool(name=f"pfg{half}_ps", bufs=2, space="PSUM")
    for m in range(MG):
        gp = pf_gps.tile([128, 256], FP32, name="gp")
        hp = pf_gps.tile([128, 256], FP32, name="hp")
        for a in range(DK // 2):
            nc.tensor.matmul(gp[:], st["wg"][m][:, 2 * a:2 * a + 2, :],
                             h2f[:, 2 * a:2 * a + 2, sl],
                             start=(a == 0), stop=(a == DK // 2 - 1),
                             perf_mode=PM_DR)
        for a in range(DK // 2):
            nc.tensor.matmul(hp[:], st["wh"][m][:, 2 * a:2 * a + 2, :],
                             h2f[:, 2 * a:2 * a + 2, sl],
                             start=(a == 0), stop=(a == DK // 2 - 1),
                             perf_mode=PM_DR)
        sg = pfg.tile([128, 256], BF16, name="sg")
        nc.scalar.activation(sg[:], gp[:], AF.Silu, scale=1.0 / 256.0)
        # ghF = sg * (hp/256) * 8  (x8 keeps ghF in fp8 range)
        nc.vector.scalar_tensor_tensor(
            st["ghF"][:, m, sl], hp[:], 1.0 / 32.0, sg[:],
            op0=ALU.mult, op1=ALU.mult)
    pfg.release()
    pf_gps.release()


def _ffn_out_pass(nc, tc, st, x1, out_d, ts_):
    """ghT @ ffn_out + gated residual for two 128-token chunks."""
    bcast = st["bcast"]
    pf = tc.alloc_tile_pool(name=f"pfo{ts_[0]}", bufs=1)
    pf_ops = tc.alloc_tile_pool(name=f"pfo{ts_[0]}_ps", bufs=1, space="PSUM")
    fps = {t: [pf_ops.tile([128, 384], FP32, name=f"fps{t}_{hf}")
               for hf in range(2)] for t in ts_}
    for p in range(MG // 2):
        for t in ts_:
            for hf in range(2):
                nc.tensor.matmul(
                    fps[t][hf][:],
                    st["ghF"][:, 2 * p:2 * p + 2, 128 * t:128 * (t + 1)],
                    st["wo_f"][:, 2 * p:2 * p + 2,
                               384 * hf:384 * (hf + 1)],
                    start=(p == 0), stop=(p == MG // 2 - 1),
                    perf_mode=PM_DR)
    for t in ts_:
        ot = pf.tile([128, D], FP32, name="ot")
        for hf in range(2):
            tt = pf.tile([128, 384], FP32, name="tt")
            # fps carries x(8*32); divide out with the gate multiply
            nc.vector.scalar_tensor_tensor(
                tt[:], fps[t][hf][:], 1.0 / 256.0,
                bcast[5][:, 384 * hf:384 * (hf + 1)],
                op0=ALU.mult, op1=ALU.mult)
            nc.vector.tensor_add(ot[:, 384 * hf:384 * (hf + 1)],
                                 tt[:],
                                 x1[t][:, 384 * hf:384 * (hf + 1)])
        nc.sync.dma_start(out_d.ap()[128 * t:128 * (t + 1), :], ot[:])
    pf.release()
    pf_ops.release()


def _emit(nc, tc, dr, out_d):
    with tc.tile_pool(name="pers", bufs=1) as pers, \
         tc.tile_pool(name="dram", bufs=1, space="DRAM") as dram:
        st = {}
        st["ident_bf"] = pers.tile([128, 128], BF16, name="ident_bf")
        make_identity(nc, st["ident_bf"][:])
        ones_f = pers.tile([1, 128], FP32, name="ones_f")
        nc.vector.memset(ones_f[:], 1.0)
        st["ones_r"] = pers.tile([1, 128], FP32R, name="ones_r")
        nc.scalar.activation(st["ones_r"][:], ones_f[:], AF.Copy)
        st["ones_bf"] = pers.tile([1, 128], BF16, name="ones_bf")
        nc.scalar.activation(st["ones_bf"][:], ones_f[:], AF.Copy)
        onescol_f = pers.tile([128, 1], FP32, name="onescol_f")
        nc.vector.memset(onescol_f[:], 1.0)
        st["onescol_bf"] = pers.tile([128, 1], BF16, name="onescol_bf")
        nc.scalar.activation(st["onescol_bf"][:], onescol_f[:], AF.Copy)
        st["ones3_f"] = pers.tile([128, 3], FP32, name="ones3_f")
        nc.vector.memset(st["ones3_f"][:], 1.0)
        st["eps_sb"] = pers.tile([128, 1], FP32, name="eps_sb")
        nc.vector.memset(st["eps_sb"][:], EPS)

        st["gsel_sb"] = pers.tile([2, 1], BF16, name="gsel_sb")
        nc.sync.dma_start(st["gsel_sb"][:], dr["gsel"].ap())
        st["mod_cols"] = pers.tile([128, 12], FP32, name="mod_cols")
        st["mod_cols_bf"] = pers.tile([128, 12], BF16, name="mod_cols_bf")
        st["bcast"] = {m: pers.tile([128, D], BF16, name=f"bcast{m}")
                       for m in (2, 3, 4, 5)}
        st["sq_j"] = [pers.tile([1, 512], FP32R, name=f"sq_j{j}")
                      for j in range(4)]
        st["rinv_j"] = [pers.tile([1, 512], BF16, name=f"rinv_j{j}")
                        for j in range(4)]
        st["ones3t"] = pers.tile([65, 64], FP32R, name="ones3t")
        ones3f = pers.tile([65, 64], FP32, name="ones3f")
        nc.vector.memset(ones3f[:], 1.0)
        nc.scalar.activation(st["ones3t"][:], ones3f[:], AF.Copy)
        st["rb"] = pers.tile([128, L], BF16, name="rb")
        st["rb_cols"] = pers.tile([128, 16], FP32, name="rb_cols")
        st["bq_r"] = pers.tile([1, 384], FP32R, name="bq_r")
        st["bvw_r"] = pers.tile([1, HC * HD], FP32R, name="bvw_r")
        st["ar_a_in"] = dram.tile([2, 2 * D], FP32, name="ar_a_in")
        st["ar_a_out"] = dram.tile([2, 2 * D], FP32, name="ar_a_out")
        st["ar_b_in"] = dram.tile([2, 4 * D], FP32, name="ar_b_in")
        st["ar_b_out"] = dram.tile([2, 4 * D], FP32, name="ar_b_out")
        for j in range(4):
            st[f"rs_in{j}"] = dram.tile([512, D], BF16, name=f"rs_in{j}")
            st[f"rs_out{j}"] = dram.tile([QTOK, D], BF16, name=f"rs_out{j}")

        _phase_a1(nc, tc, dr, st)

        with tc.tile_pool(name="p_x1", bufs=1) as p_x1:
            # prefetch residual slice into x1 (rs partial added in later)
            x1 = [p_x1.tile([128, D], BF16, name=f"x1_{t}") for t in range(4)]
            for t in range(4):
                nc.sync.dma_start(
                    x1[t][:], dr["x_slice"].ap()[128 * t:128 * (t + 1), :])

            p_qv = tc.alloc_tile_pool(name="p_qv", bufs=1)
            qkr_A = p_qv.tile([128, L], BF16, name="qkr_A")
            qkr_B = p_qv.tile([128, L], BF16, name="qkr_B")
            qkr_C = p_qv.tile([64, L], BF16, name="qkr_C")
            qkr_D = p_qv.tile([64, L], BF16, name="qkr_D")
            qkr_E = p_qv.tile([128, L], BF16, name="qkr_E")
            qkr_F = p_qv.tile([128, L], BF16, name="qkr_F")
            qkr = (qkr_A, qkr_B, qkr_C, qkr_D, qkr_E, qkr_F)
            v_sb = [p_qv.tile([128, 2, 240], FP8E4, name=f"v_sb{t}")
                    for t in range(KC // 2)]
            wo_sb = [p_qv.tile([64, D], BF16, name=f"wo{h}")
                     for h in range(HC)]
            for h in range(HC):
                nc.sync.dma_start(wo_sb[h][:],
                                  dr["wo"].ap()[64 * h:64 * (h + 1), :])
            wqk_sb = [p_qv.tile([128, 384], BF16, name=f"wqk{k}")
                      for k in range(DK)]
            wv_sb = [p_qv.tile([128, HC * HD], BF16, name=f"wv{k}")
                     for k in range(DK)]
            for k in range(DK):
                nc.sync.dma_start(wqk_sb[k][:],
                                  dr["wqk"].ap()[128 * k:128 * (k + 1), :])
                nc.sync.dma_start(wv_sb[k][:],
                                  dr["wv"].ap()[128 * k:128 * (k + 1), :])

            p_xt = tc.alloc_tile_pool(name="p_xt", bufs=1)
            xt = [p_xt.tile([128, L], BF16, name=f"xt{k}") for k in range(DK)]

            _phase_b_stats(nc, tc, dr, st, xt)
            _phase_a2_cols(nc, tc, dr, st)
            _phase_wmod(nc, tc, st, wqk_sb, wv_sb)
            _phase_c(nc, tc, dr, st, xt, qkr, v_sb, wqk_sb, wv_sb)
            p_xt.release()

            # FFN gate/hidden weights + ghF: loads stream during attention
            pfw = tc.alloc_tile_pool(name="pfw", bufs=1)
            st["wg"] = [pfw.tile([128, DK, 128], FP8E4, name=f"wg{m}")
                        for m in range(MG)]
            st["wh"] = [pfw.tile([128, DK, 128], FP8E4, name=f"wh{m}")
                        for m in range(MG)]
            ghF = pfw.tile([128, MG, TOK], FP8E4, name="ghF")
            st["ghF"] = ghF
            for m in range(MG):
                nc.sync.dma_start(
                    st["wg"][m][:],
                    dr["wg_blk"].ap()[128 * m:128 * (m + 1), :])
                nc.sync.dma_start(
                    st["wh"][m][:],
                    dr["wh_blk"].ap()[128 * m:128 * (m + 1), :])
            p_h2t = tc.alloc_tile_pool(name="p_h2t", bufs=1)
            h2t = p_h2t.tile([128, DK, TOK], FP8E4, name="h2f")

            pwo = tc.alloc_tile_pool(name="pwo", bufs=1)
            st["wo_f"] = pwo.tile([128, MG, D], FP8E4, name="wof")
            nc.sync.dma_start(st["wo_f"][:], dr["wout2"].ap())

            _phase_a2_bcast(nc, tc, dr, st)

            pfp0 = tc.alloc_tile_pool(name="pfp0", bufs=1)
            prep0, h2cs0 = _ffn_prep_closures(nc, tc, st, x1, 0, pfp0)

            def out1_dma(j, tc4, out1):
                nc.sync.dma_start(
                    st[f"rs_in{j}"][128 * tc4:128 * (tc4 + 1), :], out1[:])

            def fillers_for(p, trps):
                if p == 4:
                    return prep0
                if p == 5:
                    return _ffn_tr_closures(nc, tc, st, h2t, 0, h2cs0, trps)
                return []

            _attention_core(nc, tc, st, qkr, v_sb, wo_sb, out1_dma,
                            fillers_for)
            pfp0.release()

            pfp1 = tc.alloc_tile_pool(name="pfp1", bufs=1)
            prep1, h2cs1 = _ffn_prep_closures(nc, tc, st, x1, 1, pfp1)
            prep1[0]()          # t=2 residual+stats (RS-q2 already landed)
            _ffn_gatehid(nc, tc, st, h2t, 0)
            _ffn_out_pass(nc, tc, st, x1, out_d, (0, 1))
            for f in prep1[1:]:
                f()
            with tc.tile_pool(name="trp1_ps", bufs=2, space="PSUM") as trps1:
                for f in _ffn_tr_closures(nc, tc, st, h2t, 1, h2cs1, trps1):
                    f()
            pfp1.release()
            _ffn_gatehid(nc, tc, st, h2t, 1)
            _ffn_out_pass(nc, tc, st, x1, out_d, (2, 3))
            pwo.release()
            p_h2t.release()
            pfw.release()
            p_qv.release()


def _attention_core(nc, tc, st, qkr, v_sb, wo_sb, out1_dma, fillers_for):
    """Head-paired attention: 6 pairs of (j,h) streams, one at array rows
    0-63 and one at rows 64-127, whose score matmuls run concurrently as
    row tiles into the two banks of a shared wide PSUM tile. One wide exp
    [128,1024] covers both streams' chunk; PV runs as fp8 DoubleRow over
    chunk pairs. Per-quarter ReduceScatter after each j completes."""
    bcast = st["bcast"]
    qkr_A, qkr_B, qkr_C, qkr_D, qkr_E, qkr_F = qkr

    def kq(j, h, base):
        jsl = slice(512 * j, 512 * (j + 1))
        if h == 0:
            return (lambda s: qkr_B[0:64, s]), qkr_A[0:64, jsl]
        if h == 1:
            return (lambda s: qkr_B[64:128, s]), qkr_A[64:128, jsl]
        if base == 0:
            return (lambda s: qkr_D[0:64, s]), qkr_C[0:64, jsl]
        return (lambda s: qkr_F[64:128, s]), qkr_E[64:128, jsl]

    # (j, h, array-base) stream pairs; second element always at rows 64+
    PAIRS = [((0, 0, 0), (0, 1, 64)), ((0, 2, 0), (1, 1, 64)),
             ((1, 0, 0), (1, 2, 64)), ((2, 0, 0), (2, 1, 64)),
             ((2, 2, 0), (3, 1, 64)), ((3, 0, 0), (3, 2, 64))]
    # pair index after which each j is fully computed
    J_DONE = {1: 0, 2: 1, 4: 2, 5: 3}

    with tc.tile_pool(name="at", bufs=2) as at, \
         tc.tile_pool(name="at_exp", bufs=3) as at_exp, \
         tc.tile_pool(name="at_sps", bufs=2, space="PSUM") as at_sps, \
         tc.tile_pool(name="at_ops", bufs=1, space="PSUM") as at_ops, \
         tc.tile_pool(name="at_shps", bufs=2, space="PSUM") as at_shps:
        dens = {j: at.tile([65, 512], FP32, name=f"den{j}", bufs=1)
                for j in range(JT)}
        den_rs = {}
        o65 = {}
        o_sb = {}

        def emit_jnorm(j):
            """Start j's softmax-denominator reciprocal (DVE; cannot
            stall the PE queue head) ..."""
            den_r = at.tile([65, 512], FP32R, name=f"den_r{j}", bufs=1)
            nc.vector.reciprocal(den_r[:], dens[j][:])
            den_rs[j] = den_r

        def jtail_closures(j):
            """... and return the W_o / out1 / ReduceScatter emissions as
            closures, drained inside the NEXT pair so a W_o waiting on the
            normalize chain never blocks the next pair's score matmuls."""
            def norm(h):
                def run():
                    bps = at_shps.tile([128, 512], FP32, name="bps", tag="sh")
                    nc.tensor.matmul(bps[0:64, :],
                                     st["ones3t"][32 * h:32 * h + 1, :],
                                     den_rs[j][32 * h:32 * h + 1, :],
                                     start=True, stop=True)
                    rb64 = at.tile([64, 512], BF16, name="rb64")
                    nc.vector.tensor_copy(rb64[:], bps[0:64, :])
                    osb = at.tile([64, 512], BF16, name=f"osb_{j}_{h}",
                                  bufs=1)
                    nc.vector.tensor_mul(osb[:], o65[(j, h)][:], rb64[:])
                    o_sb[(j, h)] = osb
                return run

            def wo(tc4):
                def run():
                    out1 = at.tile([128, D], BF16, name="out1", bufs=2)
                    for half in range(2):
                        wps = at_shps.tile([128, 384], FP32, name="wps",
                                           tag="sh")
                        for h in range(HC):
                            nc.tensor.matmul(
                                wps[:],
                                o_sb[(j, h)][:, 128 * tc4:128 * (tc4 + 1)],
                                wo_sb[h][:, 384 * half:384 * (half + 1)],
                                start=(h == 0), stop=(h == HC - 1))
                        nc.vector.tensor_mul(
                            out1[:, 384 * half:384 * (half + 1)], wps[:],
                            bcast[2][:, 384 * half:384 * (half + 1)])
                    out1_dma(j, tc4, out1)
                return run

            def rs():
                nc.gpsimd.collective_compute(
                    "ReduceScatter", ALU.add, replica_groups=GROUPS,
                    ins=[st[f"rs_in{j}"].opt()],
                    outs=[st[f"rs_out{j}"].opt()])
            return [norm(h) for h in range(HC)] + \
                [wo(t) for t in range(4)] + [rs]

        pending = []
        for p, (sA, sB) in enumerate(PAIRS):
            fiter = iter(pending + list(fillers_for(p, at_shps)))
            pending = []
            jA, hA, _ = sA
            jB, hB, _ = sB
            kA, qA = kq(*sA)
            kB, qB = kq(*sB)
            opsA = at_ops.tile([128, 512], FP32, name="opsA")
            opsB = at_ops.tile([128, 512], FP32, name="opsB")
            exs = {}
            for dg in range(9):
                if dg < 8:
                    ex2 = at_exp.tile([128, 2, 1024], FP8E4, name="ex2")
                    for c in range(2):
                        kc = 2 * dg + c
                        ksl = slice(128 * kc, 128 * (kc + 1))
                        wide = at_sps.tile([128, 1024], FP32, name="wide")
                        nc.tensor.matmul(wide[:, 0:512], kA(ksl), qA,
                                         start=True, stop=True)
                        nc.tensor.matmul(wide[:, 512:1024], kB(ksl), qB,
                                         start=True, stop=True)
                        nc.scalar.activation(ex2[:, c, :], wide[:], AF.Exp,
                                             scale=1.0 / SCALE)
                    exs[dg] = ex2
                if dg >= 1:
                    gg = dg - 1
                    ex2 = exs.pop(gg)
                    nc.tensor.matmul(
                        opsA[0:65, :], v_sb[gg][:, :, 80 * hA:80 * hA + 65],
                        ex2[:, :, 0:512],
                        start=(gg == 0), stop=(gg == 7), perf_mode=PM_DR)
                    nc.tensor.matmul(
                        opsB[0:65, :], v_sb[gg][:, :, 80 * hB:80 * hB + 65],
                        ex2[:, :, 512:1024],
                        start=(gg == 0), stop=(gg == 7), perf_mode=PM_DR)
                    if dg >= 3:
                        for _ in range(2):
                            f = next(fiter, None)
                            if f is None:
                                break
                            f()
            for (jx, hx, ops) in ((jA, hA, opsA), (jB, hB, opsB)):
                o = at.tile([64, 512], BF16, name=f"o65_{jx}_{hx}", bufs=1)
                nc.vector.tensor_copy(o[:], ops[0:64, :])
                o65[(jx, hx)] = o
                nc.vector.tensor_copy(dens[jx][32 * hx:32 * hx + 1, :],
                                      ops[64:65, :])
            if p in J_DONE:
                jd = J_DONE[p]
                emit_jnorm(jd)
                if p < 5:
                    pending = jtail_closures(jd)
                else:
                    for f in jtail_closures(jd):
                        f()
            for f in fiter:
                f()


# ---------------------------------------------------------------- entry
def get_program(reps=1):
    key = f"nc{reps}"
    if key not in _CACHE:
        _CACHE[key] = build_program(reps)
    return _CACHE[key]


def make_in_maps(inputs):
    cores = host_prep(inputs)
    names = [s[0] for s in DRAM_SPECS]
    return [{n: cores[i][n] for n in names} for i in range(NC_)]


def kernel(**inputs):
    nc = get_program()
    in_maps = make_in_maps(inputs)
    res = bass_utils.run_bass_kernel_spmd(nc, in_maps, list(range(NC_)))
    out = np.zeros((B, L, D), np.float32)
    for i in range(NC_):
        g, r = i // G, i % G
        o = res.results[i]["out"]
        for t in range(4):
            out[g, 512 * t + QTOK * r: 512 * t + QTOK * (r + 1)] = \
                o[QTOK * t:QTOK * (t + 1)]
    return out
